# revision 22
# baseline (speedup 1.0000x reference)
"""Longformer self-attention Bass kernel for 8 Trainium2 NeuronCores.

Sharding: data-parallel over batch (2) x sequence-parallel (4 chunks of 1024
queries). Each core receives a transposed x slab covering its rows +-W halo
(zero padded at sequence edges), the 64 global rows, and the full weight set.

On-device layout choices (restructured for PE density / ACT batching):
  - "T layout" [E_out on partitions, seq on free] for q/k so the banded score
    matmuls need no transposes; head pairs share 128-partition slices so the
    K=64 score matmuls of the even/odd head run in distinct PE row groups
    (concurrent execution).
  - Phase 2 works in units of (head pair, 2-query-tile group): the 7 band +
    global score matmuls of one head land in a single 3-bank PSUM region
    [128, 1536] laid out in 256-col bank-aligned slots, so ONE Exp covers
    the whole unit (amortizes the ~293ns ACTIVATE overhead).
  - PV is computed transposed: v chunks [128, 65] are the stationary operand
    (cheap LDWEIGHTS) and the exp'd scores stream as the moving operand with
    N up to 256; partial products accumulate per-element (has_written bits)
    into a [65, 256] PSUM bank. Output is [d, q], staged per head and DMA'd
    once per head; the host normalizes by the ones-column denominator
    (row 64) and transposes back.
  - kgT/vgext projections (only needed by phase 3) run in fp8e4 DoubleRow
    (weights pre-scaled x64 against subnormals, descaled 1/64 at the PSUM
    copy; ~2e-3 l2 impact, confined to the 64 global output rows) and are
    emitted as filler matmul chains inside phase 2 so the tensor engine
    never idles long enough for the HAM clock gate to re-throttle it.
  - Startup: x/wk/wq DMAs are split per k-chunk across both HWDGE queues and
    the first output chunk is computed k-outer across 4 concurrent PSUM
    accumulations, so each arriving chunk feeds 4 matmuls.
  - All masking is data-driven: key validity (padding + global-key exclusion
    from the band) is folded into the v/ones columns; band geometry needs
    only two static triangle masks; the global-score block is padded from 64
    to 128 key rows with zeroed k/v so no PSUM garbage is ever exp'd.
"""

import sys
import numpy as np

if '/opt/trn_rl_repo' not in sys.path:
    sys.path.insert(0, '/opt/trn_rl_repo')

import ml_dtypes

B, S, E, H, HD, G, W = 2, 4096, 768, 12, 64, 64, 256
NCORES = 8
CHUNK = 1024          # query rows per core
SLAB = 1536           # halo slab rows per core (CHUNK + 2W)
NQT = CHUNK // 128    # query tiles per core
NSC = SLAB // 128     # slab chunks of 128 keys
NGC = CHUNK // 128    # key chunks for the global-token partials
KE = E // 128         # contraction chunks over E
VW = HD + 1           # v head width incl. ones column
NGRP = NQT // 2       # 2-query-tile groups per core
BF16 = ml_dtypes.bfloat16

_PROGRAM = None


def _build_program():
    import concourse.bass as bass
    import concourse.mybir as mybir
    import concourse.tile as tile
    from concourse import bacc

    dt = mybir.dt
    Act = mybir.ActivationFunctionType

    nc = bacc.Bacc("TRN2", target_bir_lowering=False, debug=False,
                   num_devices=NCORES)

    xT = nc.dram_tensor("xT", [E, SLAB], dt.bfloat16, kind="ExternalInput")
    xTg = nc.dram_tensor("xTg", [E, G], dt.bfloat16, kind="ExternalInput")
    wts = {}
    for nm in ("wqt", "wkt", "wvt", "wqgt"):
        wts[nm] = nc.dram_tensor(nm, [E, E], dt.bfloat16, kind="ExternalInput")
    # fp8 (DoubleRow-interleaved) operands for the kg/vg projections, which
    # only affect the 64 global output rows (quantization impact ~2e-3 l2)
    x8_d = nc.dram_tensor("x8", [128, KE // 2, 2, CHUNK], dt.float8e4,
                          kind="ExternalInput")
    w8kg_d = nc.dram_tensor("w8kg", [128, KE // 2, 2, E], dt.float8e4,
                            kind="ExternalInput")
    w8vg_d = nc.dram_tensor("w8vg", [128, KE // 2, 2, E], dt.float8e4,
                            kind="ExternalInput")
    vmask_d = nc.dram_tensor("vmask", [128, NSC], dt.float32, kind="ExternalInput")
    vones_d = nc.dram_tensor("vones", [128, NSC, H], dt.bfloat16, kind="ExternalInput")
    gmask_d = nc.dram_tensor("gmask", [G, 1], dt.float32, kind="ExternalInput")
    gones_d = nc.dram_tensor("gones", [G, H], dt.bfloat16, kind="ExternalInput")
    tril_d = nc.dram_tensor("tril", [128, 128], dt.bfloat16, kind="ExternalInput")
    triu_d = nc.dram_tensor("triu", [128, 128], dt.bfloat16, kind="ExternalInput")
    obT = nc.dram_tensor("obT", [H, VW, CHUNK], dt.float32, kind="ExternalOutput")
    gpT = nc.dram_tensor("gpT", [VW, H * G], dt.float32, kind="ExternalOutput")

    with tile.TileContext(nc) as tc:
        with tc.tile_pool(name="main", bufs=1) as mp, \
             tc.tile_pool(name="psum", bufs=1, space="PSUM") as pp:

            # ---- resident SBUF tensors -------------------------------------
            xT_sb = mp.tile([128, KE, SLAB], dt.bfloat16)
            xTg_sb = mp.tile([128, KE, G], dt.bfloat16)
            w_sb = {nm: mp.tile([128, KE, E], dt.bfloat16, name=f"w_{nm}")
                    for nm in wts}
            qT = mp.tile([128, KE, CHUNK], dt.bfloat16)
            kT = mp.tile([128, KE, SLAB], dt.bfloat16)
            kgT = mp.tile([128, KE, CHUNK], dt.bfloat16)
            # global-key scores: stationary padded to 128 key rows (64 zeros)
            kglobT = mp.tile([128, KE, 128], dt.bfloat16)
            qgT = mp.tile([128, KE, G], dt.bfloat16)
            vext = mp.tile([128, NSC, H * VW], dt.bfloat16)
            vgext = mp.tile([128, NGC, H * VW], dt.bfloat16)
            vglob = mp.tile([128, H * VW], dt.bfloat16)
            vmask = mp.tile([128, NSC], dt.float32)
            gmask = mp.tile([G, 1], dt.float32)
            tril = mp.tile([128, 128], dt.bfloat16)
            triu = mp.tile([128, 128], dt.bfloat16)

            x8_sb = mp.tile([128, KE // 2, 2, CHUNK], dt.float8e4)
            w8kg_sb = mp.tile([128, KE // 2, 2, E], dt.float8e4)
            w8vg_sb = mp.tile([128, KE // 2, 2, E], dt.float8e4)

            nc.vector.memset(kglobT[:, :, :], 0.0)
            nc.vector.memset(vglob[:, :], 0.0)

            # x, wk, wq split per k-chunk so the first kT/qT matmuls start
            # ~2us in; inputs alternate over the two HWDGE queues (SP + ACT)
            # so x and weights stream in parallel
            xT_r = xT.rearrange("(c p) n -> p c n", p=128)
            wkt_r = wts["wkt"].rearrange("(c p) n -> p c n", p=128)
            wqt_r = wts["wqt"].rearrange("(c p) n -> p c n", p=128)
            for c in range(KE):
                nc.scalar.dma_start(xT_sb[:, c, :], xT_r[:, c, :])
                nc.sync.dma_start(w_sb["wkt"][:, c, :], wkt_r[:, c, :])
                nc.sync.dma_start(w_sb["wqt"][:, c, :], wqt_r[:, c, :])
            nc.scalar.dma_start(xTg_sb[:, :, :], xTg.rearrange("(c p) n -> p c n", p=128))
            nc.sync.dma_start(w_sb["wvt"][:, :, :],
                              wts["wvt"].rearrange("(c p) n -> p c n", p=128))
            nc.scalar.dma_start(w_sb["wqgt"][:, :, :],
                                wts["wqgt"].rearrange("(c p) n -> p c n", p=128))
            nc.scalar.dma_start(x8_sb[:, :, :, :], x8_d[:, :, :, :])
            nc.scalar.dma_start(w8kg_sb[:, :, :, :], w8kg_d[:, :, :, :])
            nc.scalar.dma_start(w8vg_sb[:, :, :, :], w8vg_d[:, :, :, :])
            nc.sync.dma_start(vmask[:, :], vmask_d[:, :])
            nc.sync.dma_start(gmask[:, :], gmask_d[:, :])
            nc.sync.dma_start(tril[:, :], tril_d[:, :])
            nc.sync.dma_start(triu[:, :], triu_d[:, :])
            # ones columns (gated by key validity)
            nc.sync.dma_start(
                vext.rearrange("p c (h e) -> p c h e", e=VW)[:, :, :, HD:HD + 1],
                vones_d.rearrange("p c (h o) -> p c h o", o=1))
            nc.sync.dma_start(
                vglob.rearrange("p (h e) -> p h e", e=VW)[0:G, :, HD:HD + 1],
                gones_d.rearrange("p (h o) -> p h o", o=1))
            nc.vector.memset(
                vgext.rearrange("p c (h e) -> p c h e", e=VW)[:, :, :, HD:HD + 1], 1.0)

            # ---- phase 1: projections (PSUM tag "mm", copies on DVE) -------
            def proj_T(dst, wname, src, src_off, n_total, m_list=None):
                # dst[e_out, n] = sum_e w[e, e_out] * src[e, src_off + n]
                for m in (range(KE) if m_list is None else m_list):
                    n0 = 0
                    while n0 < n_total:
                        nw = min(512, n_total - n0)
                        ps = pp.tile([128, 512], dt.float32, tag="mm", bufs=2,
                                     name="ps_projT")
                        for k in range(KE):
                            nc.tensor.matmul(
                                ps[:, :nw],
                                w_sb[wname][:, k, m * 128:(m + 1) * 128],
                                src[:, k, src_off + n0: src_off + n0 + nw],
                                start=(k == 0), stop=(k == KE - 1))
                        nc.vector.tensor_copy(dst[:, m, n0:n0 + nw],
                                              ps[:, :nw])
                        n0 += nw

            def proj_V_chain(dst, wname, src, src_off, sI, h0, nw, mask_tile):
                # dst[seq, h, d] = sum_e src[e, seq] * w[e, 64h+d], * mask[seq]
                ps = pp.tile([128, 512], dt.float32, tag="mm", bufs=2,
                             name="ps_projV")
                for k in range(KE):
                    nc.tensor.matmul(
                        ps[:, :nw],
                        src[:, k, src_off + sI * 128: src_off + (sI + 1) * 128],
                        w_sb[wname][:, k, h0:h0 + nw],
                        start=(k == 0), stop=(k == KE - 1))
                dv = dst.rearrange("p c (h e) -> p c h e", e=VW)[
                    :, sI, h0 // HD:(h0 + nw) // HD, 0:HD]
                pv = ps[:, :nw].rearrange("p (h e) -> p h e", e=HD)
                if mask_tile is None:
                    nc.vector.tensor_copy(dv, pv)
                else:
                    nc.vector.tensor_scalar_mul(dv, pv, mask_tile[:, sI:sI + 1])

            # boot: m=0 projections k-outer across 4 concurrent PSUM tiles
            # so each arriving x/w DMA chunk feeds 4 matmuls (fills the
            # DMA-paced startup window); the sc pools are idle here
            boot = [
                (pp.tile([128, 512], dt.float32, tag="mm", bufs=2, name="bootA"),
                 "wkt", 0),
                (pp.tile([128, 512], dt.float32, tag="mm", bufs=2, name="bootB"),
                 "wkt", 512),
                (pp.tile([128, 512], dt.float32, tag="scE", bufs=1, name="bootC"),
                 "wkt", 1024),
                (pp.tile([128, 512], dt.float32, tag="scO", bufs=1, name="bootD"),
                 "wqt", W),
            ]
            for k in range(KE):
                for ps, wname, n0 in boot:
                    nc.tensor.matmul(
                        ps[:, :], w_sb[wname][:, k, 0:128],
                        xT_sb[:, k, n0:n0 + 512],
                        start=(k == 0), stop=(k == KE - 1))
            for ps, wname, n0 in boot:
                dst = kT if wname == "wkt" else qT
                nc.vector.tensor_copy(dst[:, 0, (n0 - W if wname == "wqt" else n0):
                                          (n0 - W if wname == "wqt" else n0) + 512],
                                      ps[:, :])
            psq = pp.tile([128, 512], dt.float32, tag="mm", bufs=2,
                          name="ps_bootq")
            for k in range(KE):
                nc.tensor.matmul(psq[:, :], w_sb["wqt"][:, k, 0:128],
                                 xT_sb[:, k, W + 512:W + 1024],
                                 start=(k == 0), stop=(k == KE - 1))
            nc.vector.tensor_copy(qT[:, 0, 512:1024], psq[:, :])

            # preload: everything phase 2 depends on (kT/qT interleaved
            # per m-chunk so PE can fill DMA-arrival stalls)
            for m in range(1, KE):
                proj_T(kT, "wkt", xT_sb, 0, SLAB, m_list=[m])
                proj_T(qT, "wqt", xT_sb, W, CHUNK, m_list=[m])
            # kglobT: Wk projection of the G global rows (cols 64:128 stay 0)
            for m in range(KE):
                psg = pp.tile([128, 512], dt.float32, tag="mm", bufs=2,
                              name="ps_kglob")
                for k in range(KE):
                    nc.tensor.matmul(
                        psg[:, 0:G],
                        w_sb["wkt"][:, k, m * 128:(m + 1) * 128],
                        xTg_sb[:, k, :], start=(k == 0), stop=(k == KE - 1))
                nc.vector.tensor_copy(kglobT[:, m, 0:G], psg[:, 0:G])
            # v of the global rows, gated by j < n_global (rows 64:128 stay 0)
            for h0, nw in ((0, 512), (512, 256)):
                psg = pp.tile([128, 512], dt.float32, tag="mm", bufs=2,
                              name="ps_vglob")
                for k in range(KE):
                    nc.tensor.matmul(
                        psg[0:G, :nw], xTg_sb[:, k, :], w_sb["wvt"][:, k, h0:h0 + nw],
                        start=(k == 0), stop=(k == KE - 1))
                dv = vglob.rearrange("p (h e) -> p h e", e=VW)[
                    0:G, h0 // HD:(h0 + nw) // HD, 0:HD]
                nc.vector.tensor_scalar_mul(
                    dv, psg[0:G, :nw].rearrange("p (h e) -> p h e", e=HD),
                    gmask[:, :])
            proj_T(qgT, "wqgt", xTg_sb, 0, G)
            for sI in range(NSC):
                for h0, nw in ((0, 512), (512, 256)):
                    proj_V_chain(vext, "wvt", xT_sb, 0, sI, h0, nw, vmask)

            # filler chains: kgT / vgext / qgT projections, only needed by
            # phase 3 — emitted one per phase-2 unit to keep the PE dense
            # while ACT runs the softmax Exps
            DR = mybir.MatmulPerfMode.DoubleRow
            W8DESCALE = 1.0 / 64.0

            def kgT_chain(m, n0, nw):
                # kgT[e_out, n] via fp8 DoubleRow: 3 matmuls of contraction 256
                ps = pp.tile([128, 512], dt.float32, tag="mm", bufs=2,
                             name="ps_fillT")
                for dch in range(KE // 2):
                    nc.tensor.matmul(
                        ps[:, :nw],
                        w8kg_sb[:, dch, :, m * 128:(m + 1) * 128],
                        x8_sb[:, dch, :, n0:n0 + nw],
                        start=(dch == 0), stop=(dch == KE // 2 - 1),
                        perf_mode=DR)
                nc.vector.tensor_scalar_mul(kgT[:, m, n0:n0 + nw], ps[:, :nw],
                                            W8DESCALE)

            def vgext_chain(sI, h0, nw):
                ps = pp.tile([128, 512], dt.float32, tag="mm", bufs=2,
                             name="ps_fillV")
                for dch in range(KE // 2):
                    nc.tensor.matmul(
                        ps[:, :nw],
                        x8_sb[:, dch, :, sI * 128:(sI + 1) * 128],
                        w8vg_sb[:, dch, :, h0:h0 + nw],
                        start=(dch == 0), stop=(dch == KE // 2 - 1),
                        perf_mode=DR)
                dv = vgext.rearrange("p c (h e) -> p c h e", e=VW)[
                    :, sI, h0 // HD:(h0 + nw) // HD, 0:HD]
                nc.vector.tensor_scalar_mul(
                    dv, ps[:, :nw].rearrange("p (h e) -> p h e", e=HD),
                    W8DESCALE)

            filler = []
            for sI in range(NGC):
                for h0, nw in ((0, 512), (512, 256)):
                    filler.append(lambda sI=sI, h0=h0, nw=nw:
                                  vgext_chain(sI, h0, nw))
            for m in range(KE):
                for n0 in (0, 512):
                    filler.append(lambda m=m, n0=n0: kgT_chain(m, n0, 512))
            fill_i = 0

            def emit_filler():
                nonlocal fill_i
                if fill_i >= len(filler):
                    return
                filler[fill_i]()
                fill_i += 1

            # ---- phase 2: banded + global-key attention --------------------
            # unit = (head pair hp, 2-tile group g). Per head: one [128, 1536]
            # PSUM score region in 256-col slots:
            #   slot0 = [chunk 2g (tile 2g, ci=0) | chunk 2g+5 (tile 2g+1, ci=4)]
            #   slot k=1..4 = chunk 2g+k: [tile 2g | tile 2g+1]
            #   slot5 = global keys (padded to 128) x 256 queries
            def emit_scores(h, g, tag):
                hp, ho = h // 2, (h % 2) * 64
                q0 = 2 * g * 128
                sc = pp.tile([128, 1536], dt.float32, tag=tag, bufs=1,
                             name=f"ps_{tag}")
                mm = []
                mm.append((sc[:, 0:128],
                           kT[ho:ho + 64, hp, (2 * g) * 128:(2 * g + 1) * 128],
                           qT[ho:ho + 64, hp, q0:q0 + 128]))
                mm.append((sc[:, 128:256],
                           kT[ho:ho + 64, hp, (2 * g + 5) * 128:(2 * g + 6) * 128],
                           qT[ho:ho + 64, hp, q0 + 128:q0 + 256]))
                for k in range(1, 5):
                    mm.append((sc[:, 256 * k:256 * k + 256],
                               kT[ho:ho + 64, hp,
                                  (2 * g + k) * 128:(2 * g + k + 1) * 128],
                               qT[ho:ho + 64, hp, q0:q0 + 256]))
                mm.append((sc[:, 1280:1536],
                           kglobT[ho:ho + 64, hp, :],
                           qT[ho:ho + 64, hp, q0:q0 + 256]))
                return sc, mm

            def _hc(h):
                return slice(h * VW, (h + 1) * VW)

            def emit_pv(h, et, g):
                pv = pp.tile([65, 256], dt.float32, tag="mm", bufs=2,
                             name="ps_pv")
                nc.tensor.matmul(pv[:, 0:128], vext[:, 2 * g, _hc(h)],
                                 et[:, 0:128], start=True, stop=False)
                for k in range(1, 5):
                    nc.tensor.matmul(pv[:, 0:256],
                                     vext[:, 2 * g + k, _hc(h)],
                                     et[:, 256 * k:256 * k + 256],
                                     start=False, stop=False)
                nc.tensor.matmul(pv[:, 128:256], vext[:, 2 * g + 5, _hc(h)],
                                 et[:, 128:256], start=False, stop=False)
                nc.tensor.matmul(pv[:, 0:256], vglob[:, _hc(h)],
                                 et[:, 1280:1536], start=False, stop=True)
                return pv

            ob_stage = {}

            def flush_pv(hE, hO, etE, etO, g):
                for h, et in ((hE, etE), (hO, etO)):
                    pv = emit_pv(h, et, g)
                    if g == 0:
                        ob_stage[h] = mp.tile([65, CHUNK], dt.float32,
                                              tag="obst", bufs=4, name="ob_sb")
                    nc.vector.tensor_copy(
                        ob_stage[h][:, 256 * g:256 * (g + 1)], pv[0:65, 0:256])
                    if g == NGRP - 1:
                        nc.sync.dma_start(obT[h, :, :], ob_stage[h][:, :])

            prev = None       # (hE, hO, etE, etO, g)
            for hp in range(H // 2):
                hE, hO = 2 * hp, 2 * hp + 1
                for g in range(NGRP):
                    # emit all even-head score MMs before the odd head's:
                    # scE(u+1) then only waits expE(u), not expO(u) — the
                    # PE queue never stalls on the odd head's bank reuse
                    scE, mmE = emit_scores(hE, g, "scE")
                    for a in mmE:
                        nc.tensor.matmul(a[0], a[1], a[2], start=True, stop=True)
                    scO, mmO = emit_scores(hO, g, "scO")
                    for b in mmO:
                        nc.tensor.matmul(b[0], b[1], b[2], start=True, stop=True)
                    etE = mp.tile([128, 1536], dt.bfloat16, tag="etE", bufs=2,
                                  name="etE")
                    etO = mp.tile([128, 1536], dt.bfloat16, tag="etO", bufs=2,
                                  name="etO")
                    nc.scalar.activation(etE[:, :], scE[:, :], Act.Exp)
                    nc.scalar.activation(etO[:, :], scO[:, :], Act.Exp)
                    # band-geometry masks: tile 2g ci=0 (slot0 lo) + ci=4
                    # (slot4 lo); tile 2g+1 ci=0 (slot1 hi) + ci=4 (slot0 hi)
                    for et in (etE, etO):
                        nc.gpsimd.tensor_mul(et[:, 0:128], et[:, 0:128],
                                             tril[:, :])
                        nc.gpsimd.tensor_mul(et[:, 384:512], et[:, 384:512],
                                             tril[:, :])
                        nc.vector.tensor_mul(et[:, 1024:1152], et[:, 1024:1152],
                                             triu[:, :])
                        nc.vector.tensor_mul(et[:, 128:256], et[:, 128:256],
                                             triu[:, :])
                    emit_filler()
                    if prev is not None:
                        flush_pv(*prev)
                    prev = (hE, hO, etE, etO, g)

            assert prev is not None
            flush_pv(*prev)

            # ---- phase 3: global-token partials over this core's keys ------
            # leftover filler chains (kgT m4/m5 — consumed only by head pairs
            # 4-5) are interleaved after the first pairs to keep PE dense
            gp_all = mp.tile([65, H * G], dt.float32, name="gp_all")
            for hp in range(H // 2):
                if hp in (0, 1):
                    emit_filler()
                    emit_filler()
                # both heads' 8 score chunks in one 2-bank region (head E at
                # cols 0:512, head O at 512:1024) -> a single Exp per pair
                tag = "scE" if hp % 2 == 0 else "scO"
                sc3 = pp.tile([128, 1024], dt.float32, tag=tag, bufs=1,
                              name="ps_sc3")
                for c in range(NGC):
                    for ho, col in ((0, c * G), (64, 512 + c * G)):
                        nc.tensor.matmul(
                            sc3[:, col:col + G],
                            kgT[ho:ho + 64, hp, c * 128:(c + 1) * 128],
                            qgT[ho:ho + 64, hp, :], start=True, stop=True)
                eg = mp.tile([128, 1024], dt.bfloat16, tag="etE", bufs=2,
                             name="eg")
                nc.scalar.activation(eg[:, :], sc3[:, :], Act.Exp)
                for h, e0 in ((2 * hp, 0), (2 * hp + 1, 512)):
                    pg = pp.tile([65, 256], dt.float32, tag="mm", bufs=2,
                                 name="ps_pg")
                    for c in range(NGC):
                        nc.tensor.matmul(pg[:, 0:G], vgext[:, c, _hc(h)],
                                         eg[:, e0 + c * G:e0 + (c + 1) * G],
                                         start=(c == 0), stop=(c == NGC - 1))
                    nc.vector.tensor_copy(gp_all[:, h * G:(h + 1) * G],
                                          pg[0:65, 0:G])
            nc.sync.dma_start(gpT[:, :], gp_all[:, :])
            while fill_i < len(filler):
                emit_filler()

    nc.compile()
    return nc


def _get_program():
    global _PROGRAM
    if _PROGRAM is None:
        _PROGRAM = _build_program()
    return _PROGRAM


def kernel(hidden_states, Wq, bq, Wk, bk, Wv, bv, Wqg, bqg, Wkg, bkg, Wvg, bvg,
           attention_mask, n_global):
    from concourse.bass_utils import run_bass_kernel_spmd

    x = np.asarray(hidden_states, np.float32)
    am = np.asarray(attention_mask)
    ng = int(n_global)
    assert ng == G, f"kernel specialized for n_global={G}, got {ng}"
    scale = np.float32(1.0 / np.sqrt(HD))

    wT = {
        "wqt": (np.asarray(Wq, np.float32).T * scale).astype(BF16),
        "wkt": np.asarray(Wk, np.float32).T.astype(BF16),
        "wvt": np.asarray(Wv, np.float32).T.astype(BF16),
        "wqgt": (np.asarray(Wqg, np.float32).T * scale).astype(BF16),
    }

    # fp8 DoubleRow-interleaved kg/vg weights, pre-scaled x64 so the 0.02-ish
    # entries land in e4m3's normal range (descaled by 1/64 on device)
    F8 = ml_dtypes.float8_e4m3
    def dr_interleave(aT):                       # [E, n] -> [128, 3, 2, n]
        return np.ascontiguousarray(
            aT.reshape(3, 128, 2, aT.shape[1]).transpose(1, 0, 2, 3))
    wT["w8kg"] = dr_interleave(
        (np.asarray(Wkg, np.float32).T * 64.0).astype(F8))
    wT["w8vg"] = dr_interleave(
        (np.asarray(Wvg, np.float32).T * 64.0).astype(F8))
    for bias in (bq, bk, bv, bqg, bkg, bvg):
        assert not np.any(np.asarray(bias)), "nonzero biases unsupported"

    tril = np.tril(np.ones((128, 128), np.float32)).astype(BF16)
    triu = np.triu(np.ones((128, 128), np.float32)).astype(BF16)

    in_maps = []
    for core in range(NCORES):
        b, cb = divmod(core, 4)
        r0 = cb * CHUNK
        lo, hi = r0 - W, r0 + CHUNK + W
        slab = np.zeros((SLAB, E), np.float32)
        s0, s1 = max(0, lo), min(S, hi)
        slab[s0 - lo: s1 - lo] = x[b, s0:s1]
        valid = np.zeros(SLAB, np.float32)
        arange = np.arange(lo, hi)
        inb = (arange >= 0) & (arange < S)
        valid[inb] = (am[b, arange[inb]] == 0).astype(np.float32)
        gvalid = np.ones(G, np.float32)  # keys < n_global (ng == G)

        in_maps.append({
            "x8": dr_interleave(
                np.ascontiguousarray(slab[W:W + CHUNK].T).astype(F8)),
            "xT": np.ascontiguousarray(slab.T).astype(BF16),
            "xTg": np.ascontiguousarray(x[b, :G].T).astype(BF16),
            **wT,
            "vmask": np.ascontiguousarray(valid.reshape(NSC, 128).T),
            "vones": np.ascontiguousarray(
                np.repeat(valid.reshape(NSC, 128).T[:, :, None], H, axis=2)
            ).astype(BF16),
            "gmask": gvalid[:, None].copy(),
            "gones": np.repeat(gvalid[:, None], H, axis=1).astype(BF16),
            "tril": tril,
            "triu": triu,
        })

    nc = _get_program()
    globals()['_last_in_maps'] = in_maps
    res = run_bass_kernel_spmd(nc, in_maps, core_ids=list(range(NCORES)))

    out = np.empty((B, S, E), np.float32)
    for core in range(NCORES):
        b, cb = divmod(core, 4)
        obT = res.results[core]["obT"]                       # [H, VW, CHUNK]
        num, den = obT[:, :HD, :], obT[:, HD:HD + 1, :]
        out[b, cb * CHUNK:(cb + 1) * CHUNK] = (
            (num / den).transpose(2, 0, 1).reshape(CHUNK, E))
    for b in range(B):
        acc = sum(res.results[b * 4 + cb]["gpT"] for cb in range(4))
        acc = acc.reshape(VW, H, G)
        gout = acc[:HD] / acc[HD:HD + 1]                     # [HD, H, G]
        out[b, :G] = gout.transpose(2, 1, 0).reshape(G, E)
    return out


# revision 23
# speedup vs baseline: 1.1706x; 1.1706x over previous
"""Longformer self-attention Bass kernel for 8 Trainium2 NeuronCores.

Sharding: data-parallel over batch (2) x sequence-parallel (4 chunks of 1024
queries). Each core receives a transposed x slab covering its rows +-W halo
(zero padded at sequence edges), the 64 global rows, and the full weight set.

On-device layout choices (restructured for PE density / ACT batching):
  - "T layout" [E_out on partitions, seq on free] for q/k so the banded score
    matmuls need no transposes; head pairs share 128-partition slices so the
    K=64 score matmuls of the even/odd head run in distinct PE row groups
    (concurrent execution).
  - Phase 2 works in units of (head pair, 2-query-tile group): the 7 band +
    global score matmuls of one head land in a single 3-bank PSUM region
    [128, 1536] laid out in 256-col bank-aligned slots, so ONE Exp covers
    the whole unit (amortizes the ~293ns ACTIVATE overhead).
  - PV is computed transposed: v chunks [128, 65] are the stationary operand
    (cheap LDWEIGHTS) and the exp'd scores stream as the moving operand with
    N up to 256; partial products accumulate per-element (has_written bits)
    into a [65, 256] PSUM bank. Output is [d, q], staged per head and DMA'd
    once per head; the host normalizes by the ones-column denominator
    (row 64) and transposes back.
  - kgT/vgext projections (only needed by phase 3) run in fp8e4 DoubleRow
    (weights pre-scaled x64 against subnormals, descaled 1/64 at the PSUM
    copy; ~2e-3 l2 impact, confined to the 64 global output rows) and are
    emitted as filler matmul chains inside phase 2 so the tensor engine
    never idles long enough for the HAM clock gate to re-throttle it.
  - Startup: x/wk/wq DMAs are split per k-chunk across both HWDGE queues and
    the first output chunk is computed k-outer across 4 concurrent PSUM
    accumulations, so each arriving chunk feeds 4 matmuls.
  - All masking is data-driven: key validity (padding + global-key exclusion
    from the band) is folded into the v/ones columns; band geometry needs
    only two static triangle masks; the global-score block is padded from 64
    to 128 key rows with zeroed k/v so no PSUM garbage is ever exp'd.
"""

import sys
import numpy as np

if '/opt/trn_rl_repo' not in sys.path:
    sys.path.insert(0, '/opt/trn_rl_repo')

import ml_dtypes

B, S, E, H, HD, G, W = 2, 4096, 768, 12, 64, 64, 256
NCORES = 8
CHUNK = 1024          # query rows per core
SLAB = 1536           # halo slab rows per core (CHUNK + 2W)
NQT = CHUNK // 128    # query tiles per core
NSC = SLAB // 128     # slab chunks of 128 keys
NGC = CHUNK // 128    # key chunks for the global-token partials
KE = E // 128         # contraction chunks over E
VW = HD + 1           # v head width incl. ones column
NGRP = NQT // 2       # 2-query-tile groups per core
BF16 = ml_dtypes.bfloat16

_PROGRAM = None


def _build_program():
    import concourse.bass as bass
    import concourse.mybir as mybir
    import concourse.tile as tile
    from concourse import bacc

    dt = mybir.dt
    Act = mybir.ActivationFunctionType

    nc = bacc.Bacc("TRN2", target_bir_lowering=False, debug=False,
                   num_devices=NCORES)

    xT = nc.dram_tensor("xT", [E, SLAB], dt.bfloat16, kind="ExternalInput")
    xTg = nc.dram_tensor("xTg", [E, G], dt.bfloat16, kind="ExternalInput")
    wts = {}
    for nm in ("wqt", "wkt", "wvt", "wqgt"):
        wts[nm] = nc.dram_tensor(nm, [E, E], dt.bfloat16, kind="ExternalInput")
    # fp8 (DoubleRow-interleaved) operands for the kg/vg projections, which
    # only affect the 64 global output rows (quantization impact ~2e-3 l2)
    x8_d = nc.dram_tensor("x8", [128, KE // 2, 2, CHUNK], dt.float8e4,
                          kind="ExternalInput")
    w8kg_d = nc.dram_tensor("w8kg", [128, KE // 2, 2, E], dt.float8e4,
                            kind="ExternalInput")
    w8vg_d = nc.dram_tensor("w8vg", [128, KE // 2, 2, E], dt.float8e4,
                            kind="ExternalInput")
    vmask_d = nc.dram_tensor("vmask", [128, NSC], dt.float32, kind="ExternalInput")
    vones_d = nc.dram_tensor("vones", [128, NSC, H], dt.bfloat16, kind="ExternalInput")
    gmask_d = nc.dram_tensor("gmask", [G, 1], dt.float32, kind="ExternalInput")
    gones_d = nc.dram_tensor("gones", [G, H], dt.bfloat16, kind="ExternalInput")
    tril_d = nc.dram_tensor("tril", [128, 128], dt.bfloat16, kind="ExternalInput")
    triu_d = nc.dram_tensor("triu", [128, 128], dt.bfloat16, kind="ExternalInput")
    obT = nc.dram_tensor("obT", [H, VW, CHUNK], dt.float32, kind="ExternalOutput")
    gpT = nc.dram_tensor("gpT", [VW, H * G], dt.float32, kind="ExternalOutput")

    with tile.TileContext(nc) as tc:
        with tc.tile_pool(name="main", bufs=1) as mp, \
             tc.tile_pool(name="psum", bufs=1, space="PSUM") as pp:

            # ---- resident SBUF tensors -------------------------------------
            xT_sb = mp.tile([128, KE, SLAB], dt.bfloat16)
            xTg_sb = mp.tile([128, KE, G], dt.bfloat16)
            w_sb = {nm: mp.tile([128, KE, E], dt.bfloat16, name=f"w_{nm}")
                    for nm in wts}
            qT = mp.tile([128, KE, CHUNK], dt.bfloat16)
            kT = mp.tile([128, KE, SLAB], dt.bfloat16)
            kgT = mp.tile([128, KE, CHUNK], dt.bfloat16)
            # global-key scores: stationary padded to 128 key rows (64 zeros)
            kglobT = mp.tile([128, KE, 128], dt.bfloat16)
            qgT = mp.tile([128, KE, G], dt.bfloat16)
            vext = mp.tile([128, NSC, H * VW], dt.bfloat16)
            vgext = mp.tile([128, NGC, H * VW], dt.bfloat16)
            vglob = mp.tile([128, H * VW], dt.bfloat16)
            vmask = mp.tile([128, NSC], dt.float32)
            gmask = mp.tile([G, 1], dt.float32)
            tril = mp.tile([128, 128], dt.bfloat16)
            triu = mp.tile([128, 128], dt.bfloat16)

            x8_sb = mp.tile([128, KE // 2, 2, CHUNK], dt.float8e4)
            w8kg_sb = mp.tile([128, KE // 2, 2, E], dt.float8e4)
            w8vg_sb = mp.tile([128, KE // 2, 2, E], dt.float8e4)

            nc.vector.memset(kglobT[:, :, :], 0.0)
            nc.vector.memset(vglob[:, :], 0.0)

            # x, wk, wq split per k-chunk so the first kT/qT matmuls start
            # ~2us in; inputs alternate over the two HWDGE queues (SP + ACT)
            # so x and weights stream in parallel
            xT_r = xT.rearrange("(c p) n -> p c n", p=128)
            wkt_r = wts["wkt"].rearrange("(c p) n -> p c n", p=128)
            wqt_r = wts["wqt"].rearrange("(c p) n -> p c n", p=128)
            for c in range(KE):
                nc.scalar.dma_start(xT_sb[:, c, :], xT_r[:, c, :])
                nc.sync.dma_start(w_sb["wkt"][:, c, :], wkt_r[:, c, :])
                nc.sync.dma_start(w_sb["wqt"][:, c, :], wqt_r[:, c, :])
            nc.scalar.dma_start(xTg_sb[:, :, :], xTg.rearrange("(c p) n -> p c n", p=128))
            nc.sync.dma_start(w_sb["wvt"][:, :, :],
                              wts["wvt"].rearrange("(c p) n -> p c n", p=128))
            nc.scalar.dma_start(w_sb["wqgt"][:, :, :],
                                wts["wqgt"].rearrange("(c p) n -> p c n", p=128))
            nc.scalar.dma_start(x8_sb[:, :, :, :], x8_d[:, :, :, :])
            nc.scalar.dma_start(w8kg_sb[:, :, :, :], w8kg_d[:, :, :, :])
            nc.scalar.dma_start(w8vg_sb[:, :, :, :], w8vg_d[:, :, :, :])
            nc.sync.dma_start(vmask[:, :], vmask_d[:, :])
            nc.sync.dma_start(gmask[:, :], gmask_d[:, :])
            nc.sync.dma_start(tril[:, :], tril_d[:, :])
            nc.sync.dma_start(triu[:, :], triu_d[:, :])
            # ones columns (gated by key validity)
            nc.sync.dma_start(
                vext.rearrange("p c (h e) -> p c h e", e=VW)[:, :, :, HD:HD + 1],
                vones_d.rearrange("p c (h o) -> p c h o", o=1))
            nc.sync.dma_start(
                vglob.rearrange("p (h e) -> p h e", e=VW)[0:G, :, HD:HD + 1],
                gones_d.rearrange("p (h o) -> p h o", o=1))
            nc.vector.memset(
                vgext.rearrange("p c (h e) -> p c h e", e=VW)[:, :, :, HD:HD + 1], 1.0)

            # ---- phase 1: projections (PSUM tag "mm", copies on DVE) -------
            def proj_T(dst, wname, src, src_off, n_total, m_list=None):
                # dst[e_out, n] = sum_e w[e, e_out] * src[e, src_off + n]
                for m in (range(KE) if m_list is None else m_list):
                    n0 = 0
                    while n0 < n_total:
                        nw = min(512, n_total - n0)
                        ps = pp.tile([128, 512], dt.float32, tag="mm", bufs=2,
                                     name="ps_projT")
                        for k in range(KE):
                            nc.tensor.matmul(
                                ps[:, :nw],
                                w_sb[wname][:, k, m * 128:(m + 1) * 128],
                                src[:, k, src_off + n0: src_off + n0 + nw],
                                start=(k == 0), stop=(k == KE - 1))
                        nc.vector.tensor_copy(dst[:, m, n0:n0 + nw],
                                              ps[:, :nw])
                        n0 += nw

            def proj_V_chain(dst, wname, src, src_off, sI, h0, nw, mask_tile):
                # dst[seq, h, d] = sum_e src[e, seq] * w[e, 64h+d], * mask[seq]
                ps = pp.tile([128, 512], dt.float32, tag="mm", bufs=2,
                             name="ps_projV")
                for k in range(KE):
                    nc.tensor.matmul(
                        ps[:, :nw],
                        src[:, k, src_off + sI * 128: src_off + (sI + 1) * 128],
                        w_sb[wname][:, k, h0:h0 + nw],
                        start=(k == 0), stop=(k == KE - 1))
                dv = dst.rearrange("p c (h e) -> p c h e", e=VW)[
                    :, sI, h0 // HD:(h0 + nw) // HD, 0:HD]
                pv = ps[:, :nw].rearrange("p (h e) -> p h e", e=HD)
                if mask_tile is None:
                    nc.vector.tensor_copy(dv, pv)
                else:
                    nc.vector.tensor_scalar_mul(dv, pv, mask_tile[:, sI:sI + 1])

            # boot: m=0 projections k-outer across 4 concurrent PSUM tiles
            # so each arriving x/w DMA chunk feeds 4 matmuls (fills the
            # DMA-paced startup window); the sc pools are idle here
            boot = [
                (pp.tile([128, 512], dt.float32, tag="mm", bufs=2, name="bootA"),
                 "wkt", 0),
                (pp.tile([128, 512], dt.float32, tag="mm", bufs=2, name="bootB"),
                 "wkt", 512),
                (pp.tile([128, 512], dt.float32, tag="scE", bufs=1, name="bootC"),
                 "wkt", 1024),
                (pp.tile([128, 512], dt.float32, tag="scO", bufs=1, name="bootD"),
                 "wqt", W),
            ]
            for k in range(KE):
                for ps, wname, n0 in boot:
                    nc.tensor.matmul(
                        ps[:, :], w_sb[wname][:, k, 0:128],
                        xT_sb[:, k, n0:n0 + 512],
                        start=(k == 0), stop=(k == KE - 1))
            for ps, wname, n0 in boot:
                dst = kT if wname == "wkt" else qT
                nc.vector.tensor_copy(dst[:, 0, (n0 - W if wname == "wqt" else n0):
                                          (n0 - W if wname == "wqt" else n0) + 512],
                                      ps[:, :])
            psq = pp.tile([128, 512], dt.float32, tag="mm", bufs=2,
                          name="ps_bootq")
            for k in range(KE):
                nc.tensor.matmul(psq[:, :], w_sb["wqt"][:, k, 0:128],
                                 xT_sb[:, k, W + 512:W + 1024],
                                 start=(k == 0), stop=(k == KE - 1))
            nc.vector.tensor_copy(qT[:, 0, 512:1024], psq[:, :])

            # preload: everything phase 2 depends on (kT/qT interleaved
            # per m-chunk so PE can fill DMA-arrival stalls)
            for m in range(1, KE):
                proj_T(kT, "wkt", xT_sb, 0, SLAB, m_list=[m])
                proj_T(qT, "wqt", xT_sb, W, CHUNK, m_list=[m])
            # kglobT: Wk projection of the G global rows (cols 64:128 stay 0)
            for m in range(KE):
                psg = pp.tile([128, 512], dt.float32, tag="mm", bufs=2,
                              name="ps_kglob")
                for k in range(KE):
                    nc.tensor.matmul(
                        psg[:, 0:G],
                        w_sb["wkt"][:, k, m * 128:(m + 1) * 128],
                        xTg_sb[:, k, :], start=(k == 0), stop=(k == KE - 1))
                nc.vector.tensor_copy(kglobT[:, m, 0:G], psg[:, 0:G])
            # v of the global rows, gated by j < n_global (rows 64:128 stay 0)
            for h0, nw in ((0, 512), (512, 256)):
                psg = pp.tile([128, 512], dt.float32, tag="mm", bufs=2,
                              name="ps_vglob")
                for k in range(KE):
                    nc.tensor.matmul(
                        psg[0:G, :nw], xTg_sb[:, k, :], w_sb["wvt"][:, k, h0:h0 + nw],
                        start=(k == 0), stop=(k == KE - 1))
                dv = vglob.rearrange("p (h e) -> p h e", e=VW)[
                    0:G, h0 // HD:(h0 + nw) // HD, 0:HD]
                nc.vector.tensor_scalar_mul(
                    dv, psg[0:G, :nw].rearrange("p (h e) -> p h e", e=HD),
                    gmask[:, :])
            proj_T(qgT, "wqgt", xTg_sb, 0, G)
            for sI in range(NSC):
                for h0, nw in ((0, 512), (512, 256)):
                    proj_V_chain(vext, "wvt", xT_sb, 0, sI, h0, nw, vmask)

            # filler chains: kgT / vgext / qgT projections, only needed by
            # phase 3 — emitted one per phase-2 unit to keep the PE dense
            # while ACT runs the softmax Exps
            DR = mybir.MatmulPerfMode.DoubleRow
            W8DESCALE = 1.0 / 64.0

            def kgT_chain(m, n0, nw):
                # kgT[e_out, n] via fp8 DoubleRow: 3 matmuls of contraction 256
                ps = pp.tile([128, 512], dt.float32, tag="mm", bufs=2,
                             name="ps_fillT")
                for dch in range(KE // 2):
                    nc.tensor.matmul(
                        ps[:, :nw],
                        w8kg_sb[:, dch, :, m * 128:(m + 1) * 128],
                        x8_sb[:, dch, :, n0:n0 + nw],
                        start=(dch == 0), stop=(dch == KE // 2 - 1),
                        perf_mode=DR)
                nc.vector.tensor_scalar_mul(kgT[:, m, n0:n0 + nw], ps[:, :nw],
                                            W8DESCALE)

            def vgext_chain(sI, h0, nw):
                ps = pp.tile([128, 512], dt.float32, tag="mm", bufs=2,
                             name="ps_fillV")
                for dch in range(KE // 2):
                    nc.tensor.matmul(
                        ps[:, :nw],
                        x8_sb[:, dch, :, sI * 128:(sI + 1) * 128],
                        w8vg_sb[:, dch, :, h0:h0 + nw],
                        start=(dch == 0), stop=(dch == KE // 2 - 1),
                        perf_mode=DR)
                dv = vgext.rearrange("p c (h e) -> p c h e", e=VW)[
                    :, sI, h0 // HD:(h0 + nw) // HD, 0:HD]
                nc.vector.tensor_scalar_mul(
                    dv, ps[:, :nw].rearrange("p (h e) -> p h e", e=HD),
                    W8DESCALE)

            filler = []
            for sI in range(NGC):
                for h0, nw in ((0, 512), (512, 256)):
                    filler.append(lambda sI=sI, h0=h0, nw=nw:
                                  vgext_chain(sI, h0, nw))
            for m in range(KE):
                for n0 in (0, 512):
                    filler.append(lambda m=m, n0=n0: kgT_chain(m, n0, 512))
            fill_i = 0

            def emit_filler():
                nonlocal fill_i
                if fill_i >= len(filler):
                    return
                filler[fill_i]()
                fill_i += 1

            # ---- phase 2: banded + global-key attention --------------------
            # unit = (head pair hp, 2-tile group g). Per head: one [128, 1536]
            # PSUM score region in 256-col slots:
            #   slot0 = [chunk 2g (tile 2g, ci=0) | chunk 2g+5 (tile 2g+1, ci=4)]
            #   slot k=1..4 = chunk 2g+k: [tile 2g | tile 2g+1]
            #   slot5 = global keys (padded to 128) x 256 queries
            def emit_scores(h, g, tag):
                hp, ho = h // 2, (h % 2) * 64
                q0 = 2 * g * 128
                sc = pp.tile([128, 1536], dt.float32, tag=tag, bufs=1,
                             name=f"ps_{tag}")
                mm = []
                mm.append((sc[:, 0:128],
                           kT[ho:ho + 64, hp, (2 * g) * 128:(2 * g + 1) * 128],
                           qT[ho:ho + 64, hp, q0:q0 + 128]))
                mm.append((sc[:, 128:256],
                           kT[ho:ho + 64, hp, (2 * g + 5) * 128:(2 * g + 6) * 128],
                           qT[ho:ho + 64, hp, q0 + 128:q0 + 256]))
                for k in range(1, 5):
                    mm.append((sc[:, 256 * k:256 * k + 256],
                               kT[ho:ho + 64, hp,
                                  (2 * g + k) * 128:(2 * g + k + 1) * 128],
                               qT[ho:ho + 64, hp, q0:q0 + 256]))
                mm.append((sc[:, 1280:1536],
                           kglobT[ho:ho + 64, hp, :],
                           qT[ho:ho + 64, hp, q0:q0 + 256]))
                return sc, mm

            def _hc(h):
                return slice(h * VW, (h + 1) * VW)

            def emit_pv(h, et, g):
                pv = pp.tile([65, 256], dt.float32, tag="mm", bufs=2,
                             name="ps_pv")
                nc.tensor.matmul(pv[:, 0:128], vext[:, 2 * g, _hc(h)],
                                 et[:, 0:128], start=True, stop=False)
                for k in range(1, 5):
                    nc.tensor.matmul(pv[:, 0:256],
                                     vext[:, 2 * g + k, _hc(h)],
                                     et[:, 256 * k:256 * k + 256],
                                     start=False, stop=False)
                nc.tensor.matmul(pv[:, 128:256], vext[:, 2 * g + 5, _hc(h)],
                                 et[:, 128:256], start=False, stop=False)
                nc.tensor.matmul(pv[:, 0:256], vglob[:, _hc(h)],
                                 et[:, 1280:1536], start=False, stop=True)
                return pv

            ob_stage = {}

            def flush_one(h, et, g):
                pv = emit_pv(h, et, g)
                if g == 0:
                    ob_stage[h] = mp.tile([65, CHUNK], dt.float32,
                                          tag="obst", bufs=4, name="ob_sb")
                nc.vector.tensor_copy(
                    ob_stage[h][:, 256 * g:256 * (g + 1)], pv[0:65, 0:256])
                if g == NGRP - 1:
                    nc.sync.dma_start(obT[h, :, :], ob_stage[h][:, :])

            def apply_masks(et):
                # band-geometry masks: tile 2g ci=0 (slot0 lo) + ci=4
                # (slot4 lo); tile 2g+1 ci=0 (slot1 hi) + ci=4 (slot0 hi)
                nc.gpsimd.tensor_mul(et[:, 0:128], et[:, 0:128], tril[:, :])
                nc.gpsimd.tensor_mul(et[:, 384:512], et[:, 384:512], tril[:, :])
                nc.vector.tensor_mul(et[:, 1024:1152], et[:, 1024:1152],
                                     triu[:, :])
                nc.vector.tensor_mul(et[:, 128:256], et[:, 128:256], triu[:, :])

            # head-staggered pipeline: step u pairs the even head of unit u
            # with the odd head of unit u-1, so the first score matmul of a
            # step only waits on an Exp that finished one ACT slot ago (never
            # on the previous unit's LAST Exp) while adjacent matmuls still
            # pack the two PE row groups.
            NU = (H // 2) * NGRP
            seq = [(2 * hp + p, g)
                   for hp in range(H // 2) for g in range(NGRP) for p in (0, 1)]
            Eseq = [x for x in seq if x[0] % 2 == 0]
            Oseq = [x for x in seq if x[0] % 2 == 1]
            etEh, etOh = {}, {}
            for u in range(NU + 1):
                mmE = mmO = None
                if u < NU:
                    scE, mmE = emit_scores(Eseq[u][0], Eseq[u][1], "scE")
                if u >= 1:
                    scO, mmO = emit_scores(Oseq[u - 1][0], Oseq[u - 1][1],
                                           "scO")
                for a, b in zip(mmE or [], mmO or []):
                    nc.tensor.matmul(a[0], a[1], a[2], start=True, stop=True)
                    nc.tensor.matmul(b[0], b[1], b[2], start=True, stop=True)
                if mmE is None or mmO is None:
                    for a in (mmE or mmO or []):
                        nc.tensor.matmul(a[0], a[1], a[2], start=True,
                                         stop=True)
                if u < NU:
                    etEh[u] = mp.tile([128, 1536], dt.bfloat16, tag="etE",
                                      bufs=2, name="etE")
                    nc.scalar.activation(etEh[u][:, :], scE[:, :], Act.Exp)
                    apply_masks(etEh[u])
                if u >= 1:
                    etOh[u - 1] = mp.tile([128, 1536], dt.bfloat16, tag="etO",
                                          bufs=2, name="etO")
                    nc.scalar.activation(etOh[u - 1][:, :], scO[:, :], Act.Exp)
                    apply_masks(etOh[u - 1])
                emit_filler()
                if u >= 1:
                    flush_one(Eseq[u - 1][0], etEh.pop(u - 1), Eseq[u - 1][1])
                if u >= 2:
                    flush_one(Oseq[u - 2][0], etOh.pop(u - 2), Oseq[u - 2][1])
            flush_one(Oseq[NU - 1][0], etOh.pop(NU - 1), Oseq[NU - 1][1])

            # ---- phase 3: global-token partials over this core's keys ------
            # leftover filler chains (kgT m4/m5 — consumed only by head pairs
            # 4-5) are interleaved after the first pairs to keep PE dense
            gp_all = mp.tile([65, H * G], dt.float32, name="gp_all")
            for hp in range(H // 2):
                if hp in (0, 1):
                    emit_filler()
                    emit_filler()
                for h in (2 * hp, 2 * hp + 1):
                    ho = (h % 2) * 64
                    tag = "scE" if h % 2 == 0 else "scO"
                    sc3 = pp.tile([128, 512], dt.float32, tag=tag, bufs=1,
                                  name="ps_sc3")
                    for c in range(NGC):
                        nc.tensor.matmul(
                            sc3[:, c * G:(c + 1) * G],
                            kgT[ho:ho + 64, hp, c * 128:(c + 1) * 128],
                            qgT[ho:ho + 64, hp, :], start=True, stop=True)
                    eg = mp.tile([128, 512], dt.bfloat16, tag="etE", bufs=2,
                                 name="eg")
                    nc.scalar.activation(eg[:, :], sc3[:, :], Act.Exp)
                    pg = pp.tile([65, 256], dt.float32, tag="mm", bufs=2,
                                 name="ps_pg")
                    for c in range(NGC):
                        nc.tensor.matmul(pg[:, 0:G], vgext[:, c, _hc(h)],
                                         eg[:, c * G:(c + 1) * G],
                                         start=(c == 0), stop=(c == NGC - 1))
                    nc.vector.tensor_copy(gp_all[:, h * G:(h + 1) * G],
                                          pg[0:65, 0:G])
            nc.sync.dma_start(gpT[:, :], gp_all[:, :])
            while fill_i < len(filler):
                emit_filler()

    nc.compile()
    return nc


def _get_program():
    global _PROGRAM
    if _PROGRAM is None:
        _PROGRAM = _build_program()
    return _PROGRAM


def kernel(hidden_states, Wq, bq, Wk, bk, Wv, bv, Wqg, bqg, Wkg, bkg, Wvg, bvg,
           attention_mask, n_global):
    from concourse.bass_utils import run_bass_kernel_spmd

    x = np.asarray(hidden_states, np.float32)
    am = np.asarray(attention_mask)
    ng = int(n_global)
    assert ng == G, f"kernel specialized for n_global={G}, got {ng}"
    scale = np.float32(1.0 / np.sqrt(HD))

    wT = {
        "wqt": (np.asarray(Wq, np.float32).T * scale).astype(BF16),
        "wkt": np.asarray(Wk, np.float32).T.astype(BF16),
        "wvt": np.asarray(Wv, np.float32).T.astype(BF16),
        "wqgt": (np.asarray(Wqg, np.float32).T * scale).astype(BF16),
    }

    # fp8 DoubleRow-interleaved kg/vg weights, pre-scaled x64 so the 0.02-ish
    # entries land in e4m3's normal range (descaled by 1/64 on device)
    F8 = ml_dtypes.float8_e4m3
    def dr_interleave(aT):                       # [E, n] -> [128, 3, 2, n]
        return np.ascontiguousarray(
            aT.reshape(3, 128, 2, aT.shape[1]).transpose(1, 0, 2, 3))
    wT["w8kg"] = dr_interleave(
        (np.asarray(Wkg, np.float32).T * 64.0).astype(F8))
    wT["w8vg"] = dr_interleave(
        (np.asarray(Wvg, np.float32).T * 64.0).astype(F8))
    for bias in (bq, bk, bv, bqg, bkg, bvg):
        assert not np.any(np.asarray(bias)), "nonzero biases unsupported"

    tril = np.tril(np.ones((128, 128), np.float32)).astype(BF16)
    triu = np.triu(np.ones((128, 128), np.float32)).astype(BF16)

    in_maps = []
    for core in range(NCORES):
        b, cb = divmod(core, 4)
        r0 = cb * CHUNK
        lo, hi = r0 - W, r0 + CHUNK + W
        slab = np.zeros((SLAB, E), np.float32)
        s0, s1 = max(0, lo), min(S, hi)
        slab[s0 - lo: s1 - lo] = x[b, s0:s1]
        valid = np.zeros(SLAB, np.float32)
        arange = np.arange(lo, hi)
        inb = (arange >= 0) & (arange < S)
        valid[inb] = (am[b, arange[inb]] == 0).astype(np.float32)
        gvalid = np.ones(G, np.float32)  # keys < n_global (ng == G)

        in_maps.append({
            "x8": dr_interleave(
                np.ascontiguousarray(slab[W:W + CHUNK].T).astype(F8)),
            "xT": np.ascontiguousarray(slab.T).astype(BF16),
            "xTg": np.ascontiguousarray(x[b, :G].T).astype(BF16),
            **wT,
            "vmask": np.ascontiguousarray(valid.reshape(NSC, 128).T),
            "vones": np.ascontiguousarray(
                np.repeat(valid.reshape(NSC, 128).T[:, :, None], H, axis=2)
            ).astype(BF16),
            "gmask": gvalid[:, None].copy(),
            "gones": np.repeat(gvalid[:, None], H, axis=1).astype(BF16),
            "tril": tril,
            "triu": triu,
        })

    nc = _get_program()
    globals()['_last_in_maps'] = in_maps
    res = run_bass_kernel_spmd(nc, in_maps, core_ids=list(range(NCORES)))

    out = np.empty((B, S, E), np.float32)
    for core in range(NCORES):
        b, cb = divmod(core, 4)
        obT = res.results[core]["obT"]                       # [H, VW, CHUNK]
        num, den = obT[:, :HD, :], obT[:, HD:HD + 1, :]
        out[b, cb * CHUNK:(cb + 1) * CHUNK] = (
            (num / den).transpose(2, 0, 1).reshape(CHUNK, E))
    for b in range(B):
        acc = sum(res.results[b * 4 + cb]["gpT"] for cb in range(4))
        acc = acc.reshape(VW, H, G)
        gout = acc[:HD] / acc[HD:HD + 1]                     # [HD, H, G]
        out[b, :G] = gout.transpose(2, 1, 0).reshape(G, E)
    return out
